# revision 1
# baseline (speedup 1.0000x reference)
"""ChebConv (K=6) Trainium2 kernel.

Strategy: batch-parallel across the 8 NeuronCores (B=8, one batch element per
core, zero inter-core communication; the graph/fc weights are replicated).
Per core the Chebyshev recurrence T_k = 2 L T_{k-1} - T_{k-2} runs as 5 SPMMs.

Each SPMM is a COO gather + segment-sum:
  - edges sorted by destination row, padded so every 128-row block owns a fixed
    number of 128-edge chunks (NCPB), all chunk-aligned.
  - rows of T_{k-1} are fetched from DRAM with SWDGE dma_gather (one 256B row
    per edge) into SBUF tiles with edge-on-partition layout.
  - a selection matrix S[e, r] = 2*val_e * (rowloc_e == r) is built on-chip by
    one chained DVE tensor_scalar op (iota == rowloc) * val.
  - TensorE computes psum[r, f] += S^T @ G per chunk, accumulating a whole
    128-row block in PSUM; the Chebyshev combine (psum - T_{k-2}) runs on DVE.
  - T_k is staged to DRAM (8 blocks per DMA) to serve as the next gather src.

The trailing dense fc uses PE-transpose to flip each [128, 64] block of T_k to
[64, 128] (feature-on-partition), then accumulates the 6 small matmuls
T_k^T-block @ W_k^T in PSUM (+ bias via a rank-1 ones @ b matmul).

DRAM tensors use a permuted row layout rr = (r % 128) * G + r // 128 so all
block-staging DMAs are contiguous; the host remaps gather indices and
un-permutes the output.
"""

import numpy as np
import concourse.bacc as bacc
import concourse.mybir as mybir
from concourse.tile import TileContext
from concourse.bass_utils import run_bass_kernel_spmd

P = 128
F = 64
OUT = 64
K_CHEB = 6
N_CORES = 8
WB = 16  # blocks per staging batch


def _prep_graph(rows, cols, vals, n_blocks):
    """Sort by row, pad each block to NCPB 128-edge chunks. Device layouts."""
    order = np.argsort(rows, kind="stable")
    rows_s = rows[order].astype(np.int64)
    cols_s = cols[order].astype(np.int64)
    vals_s = vals[order].astype(np.float32)
    blk = rows_s // P
    counts = np.bincount(blk, minlength=n_blocks)
    ncpb = int(-(-counts.max() // P))  # chunks per block
    slots_per_blk = ncpb * P
    starts = np.zeros(n_blocks, np.int64)
    starts[1:] = np.cumsum(counts)[:-1]
    e_slots = n_blocks * slots_per_blk
    pos = np.arange(len(rows_s)) - starts[blk] + blk * slots_per_blk

    cols_p = np.zeros(e_slots, np.int64)
    vals_p = np.zeros(e_slots, np.float32)
    rowloc_p = np.zeros(e_slots, np.float32)
    cols_p[pos] = cols_s
    vals_p[pos] = 2.0 * vals_s  # recurrence factor folded in; T1 = 0.5*psum
    rowloc_p[pos] = (rows_s % P).astype(np.float32)

    # permuted DRAM layout: logical row r -> dram row (r%128)*n_blocks + r//128
    # pad slots (the tail of each block) get idx -1 so dma_gather skips them;
    # their S column is val=0 so any stale SBUF data multiplies to zero.
    idx = (cols_p % P) * n_blocks + cols_p // P
    idx[vals_p == 0.0] = -1
    idx[pos] = (cols_s % P) * n_blocks + cols_s // P  # real edges (even val==0)
    idx_dev = np.tile(idx.astype(np.int16).reshape(-1, 16).T, (8, 1)).copy()
    nchunk = e_slots // P
    vals_dev = np.ascontiguousarray(vals_p.reshape(nchunk, P).T)
    rowloc_dev = np.ascontiguousarray(rowloc_p.reshape(nchunk, P).T)
    return idx_dev, vals_dev, rowloc_dev, ncpb, counts.astype(np.int64)


def _to_perm(x, g):
    return np.ascontiguousarray(
        x.reshape(g, P, x.shape[-1]).transpose(1, 0, 2).reshape(P * g, x.shape[-1])
    )


def _from_perm(xp, g):
    return np.ascontiguousarray(
        xp.reshape(P, g, xp.shape[-1]).transpose(1, 0, 2).reshape(P * g, xp.shape[-1])
    )


def _pview(t, g):
    """[P*g, F] dram tensor viewed as [P, g*F] (partition-major, contiguous)."""
    return t.rearrange("(p g) f -> p (g f)", p=P)


def build_kernel(n_blocks, ncpb, counts=None, do_gather=True, do_compute=True,
                 do_fc=True, single_packet=False):
    n_pad = n_blocks * P
    e_slots = n_blocks * ncpb * P
    nchunk = e_slots // P
    ge = ncpb * P  # edges per gather call (one block)
    dt = mybir.dt.float32

    nc = bacc.Bacc(None, target_bir_lowering=False)
    x_in = nc.dram_tensor("x_in", [n_pad, F], dt, kind="ExternalInput")
    idx_in = nc.dram_tensor("idx_in", [P, e_slots // 16], mybir.dt.int16,
                            kind="ExternalInput")
    vals_in = nc.dram_tensor("vals_in", [P, nchunk], dt, kind="ExternalInput")
    rowloc_in = nc.dram_tensor("rowloc_in", [P, nchunk], dt, kind="ExternalInput")
    iota_in = nc.dram_tensor("iota_in", [P, P], dt, kind="ExternalInput")
    ident_in = nc.dram_tensor("ident_in", [P, P], dt, kind="ExternalInput")
    wt_in = nc.dram_tensor("wt_in", [F, K_CHEB * OUT], dt, kind="ExternalInput")
    b_in = nc.dram_tensor("b_in", [1, OUT], dt, kind="ExternalInput")
    out_dram = nc.dram_tensor("out", [n_pad, OUT], dt, kind="ExternalOutput")

    nbatch = -(-n_blocks // WB)

    with TileContext(nc) as tc:
        with (
            tc.tile_pool(name="const", bufs=1) as cpool,
            tc.tile_pool(name="dram", bufs=1, space="DRAM") as dpool,
            tc.tile_pool(name="idxp", bufs=3) as ipool,
            tc.tile_pool(name="gbuf", bufs=4) as gpool,
            tc.tile_pool(name="spool", bufs=4) as spool,
            tc.tile_pool(name="stg", bufs=3) as stgpool,
            tc.tile_pool(name="pv2", bufs=1) as pvpool,
            tc.tile_pool(name="fcin", bufs=6) as fcpool,
            tc.tile_pool(name="ttp", bufs=3) as ttpool,
            tc.tile_pool(name="psA", bufs=3, space="PSUM") as psA,
            tc.tile_pool(name="psT", bufs=2, space="PSUM") as psT,
            tc.tile_pool(name="psF", bufs=2, space="PSUM") as psF,
        ):
            iota_sb = cpool.tile([P, P], dt, tag="iota")
            nc.sync.dma_start(iota_sb[:], iota_in[:])
            ident_sb = cpool.tile([P, P], dt, tag="ident")
            nc.sync.dma_start(ident_sb[:], ident_in[:])
            vals_sb = cpool.tile([P, nchunk], dt, tag="vals")
            nc.sync.dma_start(vals_sb[:], vals_in[:])
            rowloc_sb = cpool.tile([P, nchunk], dt, tag="rowloc")
            nc.sync.dma_start(rowloc_sb[:], rowloc_in[:])
            wt_sb = cpool.tile([F, K_CHEB * OUT], dt, tag="wt")
            nc.sync.dma_start(wt_sb[:], wt_in[:])
            b_sb = cpool.tile([1, OUT], dt, tag="bias")
            nc.sync.dma_start(b_sb[:], b_in[:])
            ones_sb = cpool.tile([1, P], dt, tag="ones")
            nc.vector.memset(ones_sb[:], 1.0)

            # persistent Chebyshev state: T_{k-2} stays in SBUF (ping-pong);
            # buffer k%2 is read (T_{k-2}) and overwritten (T_k) in place.
            tb = [
                cpool.tile([P, n_blocks, F], dt, tag="tb0", name="tb0"),
                cpool.tile([P, n_blocks, F], dt, tag="tb1", name="tb1"),
            ]
            nc.sync.dma_start(
                tb[0][:, :, :].rearrange("p b f -> p (b f)"),
                _pview(x_in, n_blocks)[:, :],
            )

            # pre-touch the gather buffer slots so pad chunks never hold
            # non-finite garbage (stale data is multiplied by S val 0)
            for _gi in range(4):
                g_init = gpool.tile([P, ncpb, F], dt, tag="g", name=f"g_init{_gi}")
                nc.gpsimd.memset(g_init[:], 0.0)

            t_dram = [x_in] + [
                dpool.tile([n_pad, F], dt, tag=f"t{k}", name=f"t{k}")
                for k in range(1, K_CHEB)
            ]

            # ---- 5 SPMM passes: T_k = (2L) T_{k-1} - T_{k-2}  (T_1 halved)
            for k in range(1, K_CHEB):
                src = t_dram[k - 1]
                prev2 = t_dram[k - 2] if k >= 2 else None
                dst = t_dram[k]
                for bt in range(nbatch):
                    b0 = bt * WB
                    nblk = min(WB, n_blocks - b0)
                    idx_t = ipool.tile([P, WB * (ge // 16)], mybir.dt.int16, tag="idx")
                    nc.sync.dma_start(
                        idx_t[:, : nblk * (ge // 16)],
                        idx_in[:, b0 * (ge // 16) : (b0 + nblk) * (ge // 16)],
                    )
                    tbc = tb[k % 2]
                    for j in range(nblk):
                        rb = b0 + j
                        g_tile = gpool.tile([P, ncpb, F], dt, tag="g")
                        cnt = ge if counts is None else int(counts[rb])
                        if do_gather and cnt > 0:
                            nc.gpsimd.dma_gather(
                                g_tile[:],
                                src[:, :],
                                idx_t[:, j * (ge // 16) : (j + 1) * (ge // 16)],
                                ge,
                                cnt,
                                F,
                                single_packet=single_packet,
                            )
                        psum = psA.tile([P, F], dt)
                        if not do_compute:
                            nc.vector.tensor_copy(out=stg_t[:, j, :], in_=g_tile[:, 0, :])
                            continue
                        for c in range(ncpb):
                            gc = rb * ncpb + c
                            s_tile = spool.tile([P, P], dt, tag="s")
                            nc.vector.tensor_scalar(
                                out=s_tile[:],
                                in0=iota_sb[:],
                                scalar1=rowloc_sb[:, gc : gc + 1],
                                scalar2=vals_sb[:, gc : gc + 1],
                                op0=mybir.AluOpType.is_equal,
                                op1=mybir.AluOpType.mult,
                            )
                            nc.tensor.matmul(
                                psum[:],
                                s_tile[:],
                                g_tile[:, c, :],
                                start=(c == 0),
                                stop=(c == ncpb - 1),
                            )
                        if k == 1:
                            nc.vector.tensor_scalar(
                                out=tbc[:, rb, :], in0=psum[:],
                                scalar1=0.5, scalar2=None,
                                op0=mybir.AluOpType.mult,
                            )
                        else:
                            nc.vector.tensor_tensor(
                                out=tbc[:, rb, :], in0=psum[:],
                                in1=tbc[:, rb, :], op=mybir.AluOpType.subtract,
                            )
                    nc.sync.dma_start(
                        _pview(dst, n_blocks)[:, b0 * F : (b0 + nblk) * F],
                        tbc[:, b0 : b0 + nblk, :].rearrange("p b f -> p (b f)"),
                    )

            # ---- fc: out[r, o] = sum_k T_k[r, :] @ W_k^T + b
            for bt in range(nbatch if do_fc else 0):
                b0 = bt * WB
                nblk = min(WB, n_blocks - b0)
                fc_t = []
                for k in range(K_CHEB):
                    t_t = fcpool.tile([P, WB, F], dt, tag="fcin")
                    nc.sync.dma_start(
                        t_t[:, :nblk, :].rearrange("p b f -> p (b f)"),
                        _pview(t_dram[k], n_blocks)[:, b0 * F : (b0 + nblk) * F],
                    )
                    fc_t.append(t_t)
                ostg_t = stgpool.tile([P, WB, OUT], dt, tag="ostg")
                for j in range(nblk):
                    fc_psum = psF.tile([P, OUT], dt)
                    for k in range(K_CHEB):
                        tps = psT.tile([F, P], dt)
                        nc.tensor.transpose(
                            out=tps[:], in_=fc_t[k][:, j, :], identity=ident_sb[:]
                        )
                        tt_sb = ttpool.tile([F, P], dt, tag="tt")
                        nc.scalar.copy(out=tt_sb[:], in_=tps[:])
                        nc.tensor.matmul(
                            fc_psum[:],
                            tt_sb[:],
                            wt_sb[:, k * OUT : (k + 1) * OUT],
                            start=(k == 0),
                            stop=False,
                        )
                    nc.tensor.matmul(
                        fc_psum[:], ones_sb[:], b_sb[:], start=False, stop=True
                    )
                    nc.vector.tensor_copy(out=ostg_t[:, j, :], in_=fc_psum[:])
                nc.sync.dma_start(
                    _pview(out_dram, n_blocks)[:, b0 * OUT : (b0 + nblk) * OUT],
                    ostg_t[:, :nblk, :].rearrange("p b f -> p (b f)"),
                )
    nc.finalize()
    return nc


def kernel(x, lap_rows, lap_cols, lap_vals, W, b):
    x = np.asarray(x, dtype=np.float32)
    lap_rows = np.asarray(lap_rows, dtype=np.int32)
    lap_cols = np.asarray(lap_cols, dtype=np.int32)
    lap_vals = np.asarray(lap_vals, dtype=np.float32)
    W = np.asarray(W, dtype=np.float32)
    b = np.asarray(b, dtype=np.float32)

    B, N, Fin = x.shape
    n_blocks = -(-N // P)
    n_pad = n_blocks * P

    idx_dev, vals_dev, rowloc_dev, ncpb, counts = _prep_graph(
        lap_rows, lap_cols, lap_vals, n_blocks
    )
    iota = np.tile(np.arange(P, dtype=np.float32), (P, 1))
    ident = np.eye(P, dtype=np.float32)
    # wt[f, k*OUT+o] = W[o, k*F+f]
    wt = np.ascontiguousarray(
        W.reshape(OUT, K_CHEB, Fin).transpose(2, 1, 0).reshape(Fin, K_CHEB * OUT)
    )

    nc = build_kernel(n_blocks, ncpb, counts)
    in_maps = []
    for c in range(B):
        xb = np.zeros((n_pad, Fin), np.float32)
        xb[:N] = x[c]
        in_maps.append(
            {
                "x_in": _to_perm(xb, n_blocks),
                "idx_in": idx_dev,
                "vals_in": vals_dev,
                "rowloc_in": rowloc_dev,
                "iota_in": iota,
                "ident_in": ident,
                "wt_in": wt,
                "b_in": b.reshape(1, OUT),
            }
        )
    res = run_bass_kernel_spmd(nc, in_maps, core_ids=list(range(N_CORES)))
    out = np.stack(
        [_from_perm(res.results[c]["out"], n_blocks)[:N] for c in range(B)]
    )
    return out.astype(np.float32)



# revision 4
# speedup vs baseline: 2.9877x; 2.9877x over previous
"""ChebConv (K=6) Trainium2 kernel.

Strategy: batch-parallel across the 8 NeuronCores (B=8, one batch element per
core, zero inter-core communication; the graph/fc weights are replicated).
Per core the Chebyshev recurrence T_k = 2 L T_{k-1} - T_{k-2} runs as 5 SPMMs.

Each SPMM is a COO gather + segment-sum:
  - edges sorted by destination row, padded so every 128-row block owns a fixed
    number of 128-edge chunks (NCPB), all chunk-aligned.
  - rows of T_{k-1} are fetched from DRAM with SWDGE dma_gather (one 256B row
    per edge) into SBUF tiles with edge-on-partition layout.
  - a selection matrix S[e, r] = 2*val_e * (rowloc_e == r) is built on-chip by
    one chained DVE tensor_scalar op (iota == rowloc) * val.
  - TensorE computes psum[r, f] += S^T @ G per chunk, accumulating a whole
    128-row block in PSUM; the Chebyshev combine (psum - T_{k-2}) runs on DVE.
  - T_k is staged to DRAM (8 blocks per DMA) to serve as the next gather src.

The trailing dense fc uses PE-transpose to flip each [128, 64] block of T_k to
[64, 128] (feature-on-partition), then accumulates the 6 small matmuls
T_k^T-block @ W_k^T in PSUM (+ bias via a rank-1 ones @ b matmul).

DRAM tensors use a permuted row layout rr = (r % 128) * G + r // 128 so all
block-staging DMAs are contiguous; the host remaps gather indices and
un-permutes the output.
"""

import numpy as np
import concourse.bacc as bacc
import concourse.mybir as mybir
from concourse.tile import TileContext
from concourse.bass_utils import run_bass_kernel_spmd

P = 128
F = 64
OUT = 64
K_CHEB = 6
N_CORES = 8
WB = 16  # blocks per staging batch


def _prep_graph(rows, cols, vals, n_blocks):
    """Sort by row, pad each block to NCPB 128-edge chunks. Device layouts."""
    order = np.argsort(rows, kind="stable")
    rows_s = rows[order].astype(np.int64)
    cols_s = cols[order].astype(np.int64)
    vals_s = vals[order].astype(np.float32)
    blk = rows_s // P
    counts = np.bincount(blk, minlength=n_blocks)
    ncpb = int(-(-counts.max() // P))  # chunks per block
    slots_per_blk = ncpb * P
    starts = np.zeros(n_blocks, np.int64)
    starts[1:] = np.cumsum(counts)[:-1]
    e_slots = n_blocks * slots_per_blk
    pos = np.arange(len(rows_s)) - starts[blk] + blk * slots_per_blk

    cols_p = np.zeros(e_slots, np.int64)
    vals_p = np.zeros(e_slots, np.float32)
    rowloc_p = np.zeros(e_slots, np.float32)
    cols_p[pos] = cols_s
    vals_p[pos] = 2.0 * vals_s  # recurrence factor folded in; T1 = 0.5*psum
    rowloc_p[pos] = (rows_s % P).astype(np.float32)

    # permuted DRAM layout: logical row r -> dram row (r%128)*n_blocks + r//128
    # pad slots (the tail of each block) get idx -1 so dma_gather skips them;
    # their S column is val=0 so any stale SBUF data multiplies to zero.
    idx = (cols_p % P) * n_blocks + cols_p // P
    idx[vals_p == 0.0] = -1
    idx[pos] = (cols_s % P) * n_blocks + cols_s // P  # real edges (even val==0)
    idx_dev = np.tile(idx.astype(np.int16).reshape(-1, 16).T, (8, 1)).copy()
    nchunk = e_slots // P
    vals_dev = np.ascontiguousarray(vals_p.reshape(nchunk, P).T)
    rowloc_dev = np.ascontiguousarray(rowloc_p.reshape(nchunk, P).T)
    return idx_dev, vals_dev, rowloc_dev, ncpb, counts.astype(np.int64)


def _to_perm(x, g):
    return np.ascontiguousarray(
        x.reshape(g, P, x.shape[-1]).transpose(1, 0, 2).reshape(P * g, x.shape[-1])
    )


def _from_perm(xp, g):
    return np.ascontiguousarray(
        xp.reshape(P, g, xp.shape[-1]).transpose(1, 0, 2).reshape(P * g, xp.shape[-1])
    )


def _pview(t, g):
    """[P*g, F] dram tensor viewed as [P, g*F] (partition-major, contiguous)."""
    return t.rearrange("(p g) f -> p (g f)", p=P)


def build_kernel(n_blocks, ncpb, counts=None, do_gather=True, do_compute=True,
                 do_fc=True, single_packet=False):
    n_pad = n_blocks * P
    e_slots = n_blocks * ncpb * P
    nchunk = e_slots // P
    ge = ncpb * P  # edges per gather call (one block)
    dt = mybir.dt.float32

    nc = bacc.Bacc(None, target_bir_lowering=False, num_swdge_queues=4)
    x_in = nc.dram_tensor("x_in", [n_pad, F], dt, kind="ExternalInput")
    idx_in = nc.dram_tensor("idx_in", [P, e_slots // 16], mybir.dt.int16,
                            kind="ExternalInput")
    vals_in = nc.dram_tensor("vals_in", [P, nchunk], dt, kind="ExternalInput")
    rowloc_in = nc.dram_tensor("rowloc_in", [P, nchunk], dt, kind="ExternalInput")
    iota_in = nc.dram_tensor("iota_in", [P, P], dt, kind="ExternalInput")
    ident_in = nc.dram_tensor("ident_in", [P, P], dt, kind="ExternalInput")
    wt_in = nc.dram_tensor("wt_in", [F, K_CHEB * OUT], dt, kind="ExternalInput")
    b_in = nc.dram_tensor("b_in", [1, OUT], dt, kind="ExternalInput")
    out_dram = nc.dram_tensor("out", [n_pad, OUT], dt, kind="ExternalOutput")

    nbatch = -(-n_blocks // WB)

    with TileContext(nc) as tc:
        with (
            tc.tile_pool(name="const", bufs=1) as cpool,
            tc.tile_pool(name="dram", bufs=1, space="DRAM") as dpool,
            tc.tile_pool(name="idxp", bufs=3) as ipool,
            tc.tile_pool(name="gbuf", bufs=4) as gpool,
            tc.tile_pool(name="spool", bufs=4) as spool,
            tc.tile_pool(name="stg", bufs=3) as stgpool,
            tc.tile_pool(name="pv2", bufs=1) as pvpool,
            tc.tile_pool(name="fcin", bufs=6) as fcpool,
            tc.tile_pool(name="ttp", bufs=3) as ttpool,
            tc.tile_pool(name="psA", bufs=3, space="PSUM") as psA,
            tc.tile_pool(name="psT", bufs=2, space="PSUM") as psT,
            tc.tile_pool(name="psF", bufs=2, space="PSUM") as psF,
        ):
            iota_sb = cpool.tile([P, P], dt, tag="iota")
            nc.sync.dma_start(iota_sb[:], iota_in[:])
            ident_sb = cpool.tile([P, P], dt, tag="ident")
            nc.sync.dma_start(ident_sb[:], ident_in[:])
            vals_sb = cpool.tile([P, nchunk], dt, tag="vals")
            nc.sync.dma_start(vals_sb[:], vals_in[:])
            rowloc_sb = cpool.tile([P, nchunk], dt, tag="rowloc")
            nc.sync.dma_start(rowloc_sb[:], rowloc_in[:])
            wt_sb = cpool.tile([F, K_CHEB * OUT], dt, tag="wt")
            nc.sync.dma_start(wt_sb[:], wt_in[:])
            b_sb = cpool.tile([1, OUT], dt, tag="bias")
            nc.sync.dma_start(b_sb[:], b_in[:])
            ones_sb = cpool.tile([1, P], dt, tag="ones")
            nc.vector.memset(ones_sb[:], 1.0)

            # persistent Chebyshev state: T_{k-2} stays in SBUF (ping-pong);
            # buffer k%2 is read (T_{k-2}) and overwritten (T_k) in place.
            tb = [
                cpool.tile([P, n_blocks, F], dt, tag="tb0", name="tb0"),
                cpool.tile([P, n_blocks, F], dt, tag="tb1", name="tb1"),
            ]
            nc.sync.dma_start(
                tb[0][:, :, :].rearrange("p b f -> p (b f)"),
                _pview(x_in, n_blocks)[:, :],
            )

            # pre-touch the gather buffer slots so pad chunks never hold
            # non-finite garbage (stale data is multiplied by S val 0)
            for _gi in range(4):
                g_init = gpool.tile([P, ncpb, F], dt, tag="g", name=f"g_init{_gi}")
                nc.gpsimd.memset(g_init[:], 0.0)

            t_dram = [x_in] + [
                dpool.tile([n_pad, F], dt, tag=f"t{k}", name=f"t{k}")
                for k in range(1, K_CHEB)
            ]

            # ---- 5 SPMM passes: T_k = (2L) T_{k-1} - T_{k-2}  (T_1 halved)
            for k in range(1, K_CHEB):
                src = t_dram[k - 1]
                prev2 = t_dram[k - 2] if k >= 2 else None
                dst = t_dram[k]
                for bt in range(nbatch):
                    b0 = bt * WB
                    nblk = min(WB, n_blocks - b0)
                    idx_t = ipool.tile([P, WB * (ge // 16)], mybir.dt.int16, tag="idx")
                    nc.sync.dma_start(
                        idx_t[:, : nblk * (ge // 16)],
                        idx_in[:, b0 * (ge // 16) : (b0 + nblk) * (ge // 16)],
                    )
                    tbc = tb[k % 2]
                    for j in range(nblk):
                        rb = b0 + j
                        g_tile = gpool.tile([P, ncpb, F], dt, tag="g")
                        cnt = ge if counts is None else int(counts[rb])
                        if do_gather and cnt > 0:
                            nc.gpsimd.dma_gather(
                                g_tile[:],
                                src[:, :],
                                idx_t[:, j * (ge // 16) : (j + 1) * (ge // 16)],
                                ge,
                                cnt,
                                F,
                                single_packet=single_packet,
                                queue_num=rb % 4,
                            )
                        psum = psA.tile([P, F], dt)
                        if not do_compute:
                            continue
                        for c in range(ncpb):
                            gc = rb * ncpb + c
                            s_tile = spool.tile([P, P], dt, tag="s")
                            nc.vector.tensor_scalar(
                                out=s_tile[:],
                                in0=iota_sb[:],
                                scalar1=rowloc_sb[:, gc : gc + 1],
                                scalar2=vals_sb[:, gc : gc + 1],
                                op0=mybir.AluOpType.is_equal,
                                op1=mybir.AluOpType.mult,
                            )
                            nc.tensor.matmul(
                                psum[:],
                                s_tile[:],
                                g_tile[:, c, :],
                                start=(c == 0),
                                stop=(c == ncpb - 1),
                            )
                        if k == 1:
                            nc.vector.tensor_scalar(
                                out=tbc[:, rb, :], in0=psum[:],
                                scalar1=0.5, scalar2=None,
                                op0=mybir.AluOpType.mult,
                            )
                        else:
                            nc.vector.tensor_tensor(
                                out=tbc[:, rb, :], in0=psum[:],
                                in1=tbc[:, rb, :], op=mybir.AluOpType.subtract,
                            )
                    nc.sync.dma_start(
                        _pview(dst, n_blocks)[:, b0 * F : (b0 + nblk) * F],
                        tbc[:, b0 : b0 + nblk, :].rearrange("p b f -> p (b f)"),
                    )

            # ---- fc: out[r, o] = sum_k T_k[r, :] @ W_k^T + b
            for bt in range(nbatch if do_fc else 0):
                b0 = bt * WB
                nblk = min(WB, n_blocks - b0)
                fc_t = []
                for k in range(K_CHEB):
                    t_t = fcpool.tile([P, WB, F], dt, tag="fcin")
                    nc.sync.dma_start(
                        t_t[:, :nblk, :].rearrange("p b f -> p (b f)"),
                        _pview(t_dram[k], n_blocks)[:, b0 * F : (b0 + nblk) * F],
                    )
                    fc_t.append(t_t)
                ostg_t = stgpool.tile([P, WB, OUT], dt, tag="ostg")
                for j in range(nblk):
                    fc_psum = psF.tile([P, OUT], dt)
                    for k in range(K_CHEB):
                        tps = psT.tile([F, P], dt)
                        nc.tensor.transpose(
                            out=tps[:], in_=fc_t[k][:, j, :], identity=ident_sb[:]
                        )
                        tt_sb = ttpool.tile([F, P], dt, tag="tt")
                        nc.scalar.copy(out=tt_sb[:], in_=tps[:])
                        nc.tensor.matmul(
                            fc_psum[:],
                            tt_sb[:],
                            wt_sb[:, k * OUT : (k + 1) * OUT],
                            start=(k == 0),
                            stop=False,
                        )
                    nc.tensor.matmul(
                        fc_psum[:], ones_sb[:], b_sb[:], start=False, stop=True
                    )
                    nc.vector.tensor_copy(out=ostg_t[:, j, :], in_=fc_psum[:])
                nc.sync.dma_start(
                    _pview(out_dram, n_blocks)[:, b0 * OUT : (b0 + nblk) * OUT],
                    ostg_t[:, :nblk, :].rearrange("p b f -> p (b f)"),
                )
    nc.finalize()
    return nc


def kernel(x, lap_rows, lap_cols, lap_vals, W, b):
    x = np.asarray(x, dtype=np.float32)
    lap_rows = np.asarray(lap_rows, dtype=np.int32)
    lap_cols = np.asarray(lap_cols, dtype=np.int32)
    lap_vals = np.asarray(lap_vals, dtype=np.float32)
    W = np.asarray(W, dtype=np.float32)
    b = np.asarray(b, dtype=np.float32)

    B, N, Fin = x.shape
    n_blocks = -(-N // P)
    n_pad = n_blocks * P

    idx_dev, vals_dev, rowloc_dev, ncpb, counts = _prep_graph(
        lap_rows, lap_cols, lap_vals, n_blocks
    )
    iota = np.tile(np.arange(P, dtype=np.float32), (P, 1))
    ident = np.eye(P, dtype=np.float32)
    # wt[f, k*OUT+o] = W[o, k*F+f]
    wt = np.ascontiguousarray(
        W.reshape(OUT, K_CHEB, Fin).transpose(2, 1, 0).reshape(Fin, K_CHEB * OUT)
    )

    nc = build_kernel(n_blocks, ncpb, counts)
    in_maps = []
    for c in range(B):
        xb = np.zeros((n_pad, Fin), np.float32)
        xb[:N] = x[c]
        in_maps.append(
            {
                "x_in": _to_perm(xb, n_blocks),
                "idx_in": idx_dev,
                "vals_in": vals_dev,
                "rowloc_in": rowloc_dev,
                "iota_in": iota,
                "ident_in": ident,
                "wt_in": wt,
                "b_in": b.reshape(1, OUT),
            }
        )
    res = run_bass_kernel_spmd(nc, in_maps, core_ids=list(range(N_CORES)))
    out = np.stack(
        [_from_perm(res.results[c]["out"], n_blocks)[:N] for c in range(B)]
    )
    return out.astype(np.float32)

